# revision 31
# baseline (speedup 1.0000x reference)
"""ConvLogicTree layer for Trainium2 (8 NeuronCores, SPMD data-parallel over batch).

Math: the 16 soft binary gates are all affine in the monomial basis
[1, a, b, a*b], so softmax-gate-mixing per tree node collapses to
    node(a, b) = kab*(a + alpha)*(b + beta) + delta
with per-(channel, node) coefficients k = softmax(w) @ C.  All coefficient
algebra (softmax, the factored form, folding each node's delta into the
next level's affine) is done host-side in f64; every device-side affine is
a scale+bias evaluated in the engine's fp32 ALU, stored fp16 (e5m10 keeps
the folded-delta magnitudes well inside 2e-2 tolerance).

Device pipeline — all DMA rides the SWDGE queues (~14ns/descriptor vs
~75ns/line on the sync/scalar HWDGE rings), and descriptor COUNT is the
scarce resource, so layouts are chosen to maximize bytes/descriptor:

  1. load x as [64 = channel, (b, 32, 32)] f32 (host pre-shards in
     channel-partition layout), cast to fp16
  2. DVE/ACT shift-copies build the 9-tap zero-padded unfold in SBUF,
     both batches per op (border zeros via tiny strip memsets)
  3. per-tap DMA writes to DRAM rows u[c*9+tap] = [b0 1024 | b1 1024]:
     64 descriptors x 4KB per tap, pipelined behind the copies
  4. eight indirect_dma_start gathers (hardware descriptor generation
     from per-(o,leaf) row offsets, [P,1] per-partition offset form; no
     gpsimd ucode library needed) pull one 4KB leaf row per output
     channel each; tree compute overlaps the transfers
  5. the 7-node tree runs on DVE (tensor_scalar/tensor_tensor, fp16 for
     the 2-byte DVE perf modes) + ACT (activation scale+bias); tail-path
     ops stay on DVE
  6. one 128-descriptor DMA writes the f32 output as [C_OUT, NB, L];
     the host transposes to [NB, C_OUT, H, W]
"""

import sys

sys.path.insert(0, "/opt/trn_rl_repo")

import numpy as np

import concourse.bass as bass
import concourse.bacc as bacc
import concourse.mybir as mybir
import concourse.tile as tile
from contextlib import ExitStack
from concourse.bass_utils import run_bass_kernel_spmd

F32 = mybir.dt.float32
F16 = mybir.dt.float16
I32 = mybir.dt.int32
AF = mybir.ActivationFunctionType
ALU = mybir.AluOpType

N_CORES = 8
B, C_IN, H, W = 16, 64, 32, 32
C_OUT = 128
NB = B // N_CORES          # batches per core
L = H * W                  # 1024 pixels
FD = NB * L                # free dim per compute op (batch-major pixels)

# gate g -> coefficients on [1, a, b, ab]
GATE_C = np.array(
    [
        [0, 0, 0, 0],    # 0
        [0, 0, 0, 1],    # ab
        [0, 1, 0, -1],   # a - ab
        [0, 1, 0, 0],    # a
        [0, 0, 1, -1],   # b - ab
        [0, 0, 1, 0],    # b
        [0, 1, 1, -2],   # a + b - 2ab
        [0, 1, 1, -1],   # a + b - ab
        [1, -1, -1, 1],  # 1 - (a+b-ab)
        [1, -1, -1, 2],  # 1 - (a+b-2ab)
        [1, 0, -1, 0],   # 1 - b
        [1, 0, -1, 1],   # 1 - b + ab
        [1, -1, 0, 0],   # 1 - a
        [1, -1, 0, 1],   # 1 - a + ab
        [1, 0, 0, -1],   # 1 - ab
        [1, 0, 0, 0],    # 1
    ],
    dtype=np.float64,
)

# tree wiring: (level, pair) -> weight row; rows overlap across levels
# (faithful to the module: gate_idx = 2**level - 1 + pair)
L0_ROWS = [0, 1, 2, 3]
L1_ROWS = [1, 2]
L2_ROW = 3

# scalar-tile column layout (see make_host_inputs)
N_SC = 22
USE_MULTI_QUEUE = True


def build_program():
    nc = bacc.Bacc("TRN2", target_bir_lowering=False, debug=False,
                   num_swdge_queues=4 if USE_MULTI_QUEUE else 1)

    x_in = nc.dram_tensor("x", [C_IN, NB * L], F16, kind="ExternalInput")
    sc_in = nc.dram_tensor("sc", [128, N_SC], F32, kind="ExternalInput")
    gi_in = nc.dram_tensor("gidx", [128, 8], I32, kind="ExternalInput")
    out_ext = nc.dram_tensor("out", [C_OUT, NB, L], F16, kind="ExternalOutput")
    # unfold scratch: row (c*9 + tap) holds [b0 1024 | b1 1024] fp16
    u_dram = nc.dram_tensor("u", [C_IN * 9, FD], F16)

    # SWDGE queue_num must equal Tile's DMASW lane % 4 (lanes round-robin in
    # Pool-DMA emission order), so queue = emission index % 4 for every
    # Pool-engine DMA below.
    qi = [0]

    def swq(inst):
        n = qi[0] % 4
        qi[0] += 1
        if USE_MULTI_QUEUE:
            inst.ins.queue = f"qPoolDynamic{n or ''}"
        return inst

    with tile.TileContext(nc) as tc, ExitStack() as ctx:
        pool = ctx.enter_context(tc.tile_pool(name="p", bufs=1))
        tmp = ctx.enter_context(tc.tile_pool(name="tmp", bufs=2))

        xb = pool.tile([C_IN, NB, 32, 32], F16)
        u9 = pool.tile([C_IN, 9, NB, 32, 32], F16)
        sc = pool.tile([128, N_SC], F32)
        gidx = pool.tile([128, 8], I32)
        lv = pool.tile([128, 8, FD], F16)
        pt = [pool.tile([128, FD], F16, name=f"P{i}", tag=f"P{i}") for i in range(4)]
        mt = [pool.tile([128, FD], F16, name=f"M{i}", tag=f"M{i}") for i in range(2)]
        ot = pool.tile([128, FD], F16)

        swq(nc.gpsimd.dma_start(
            out=xb[:].rearrange("c b r q -> c (b r q)"), in_=x_in[:]
        ))
        swq(nc.gpsimd.dma_start(out=gidx[:], in_=gi_in[:]))
        swq(nc.gpsimd.dma_start(out=sc[:], in_=sc_in[:]))

        # border zero strips for the 8 off-center taps (corners overlap, fine)
        for s in range(9):
            ki, kj = s // 3, s % 3
            dy, dx = ki - 1, kj - 1
            if dy:
                r = 0 if dy < 0 else 31
                nc.vector.memset(u9[:, s, :, r:r + 1, :], 0.0)
            if dx:
                c = 0 if dx < 0 else 31
                nc.vector.memset(u9[:, s, :, :, c:c + 1], 0.0)

        # 9 shifted zero-padded copies (both batches per op);
        # tap s=(ki,kj) reads x rows r+ki-1
        uv = u_dram[:].rearrange("(c s) f -> s c f", s=9)
        for s in range(9):
            ki, kj = s // 3, s % 3
            dy, dx = ki - 1, kj - 1
            r0, r1 = max(0, -dy), 32 - max(0, dy)
            c0, c1 = max(0, -dx), 32 - max(0, dx)
            dst = u9[:, s, :, r0:r1, c0:c1]
            src = xb[:, :, r0 + dy:r1 + dy, c0 + dx:c1 + dx]
            if s in (2, 5):
                nc.scalar.activation(dst, src, AF.Identity, bias=0.0, scale=1.0)
            else:
                nc.vector.tensor_scalar(dst, src, 0.0, None, op0=ALU.add)
            swq(nc.gpsimd.dma_start(
                out=uv[s],
                in_=u9[:, s].rearrange("c b r q -> c (b r q)"),
            ))

        # ---- gather the 8 leaves, one indirect DMA per leaf ([P,1] offsets
        # per partition, matching the proven indirect-DMA usage pattern)
        for j in range(8):
            swq(nc.gpsimd.indirect_dma_start(
                out=lv[:, j],
                out_offset=None,
                in_=u_dram[:],
                in_offset=bass.IndirectOffsetOnAxis(
                    ap=gidx[:, j:j + 1], axis=0
                ),
            ))

        # ---- tree; every affine is scale+bias with O(1)-safe fp32 ALU math
        def col(i):
            return sc[:, i:i + 1]

        half = FD // 2

        for p in range(4):
            a_ap, b_ap = lv[:, 2 * p], lv[:, 2 * p + 1]
            at = tmp.tile([128, FD], F16, tag="a")
            bt = tmp.tile([128, FD], F16, tag="b")
            # A = a*kab0 + kb0 ; Bt = b + beta0 (ACT handles A off the tail)
            if p < 3:
                nc.scalar.activation(at[:], a_ap, AF.Identity, bias=col(4 + p),
                                     scale=col(p))
            else:
                nc.vector.tensor_scalar(at[:], a_ap, col(p), col(4 + p),
                                        op0=ALU.mult, op1=ALU.add)
            if p < 2:
                nc.scalar.activation(bt[:], b_ap, AF.Identity, bias=col(8 + p),
                                     scale=1.0)
            elif p < 3:
                nc.vector.tensor_scalar(bt[:], b_ap, col(8 + p), None,
                                        op0=ALU.add)
            else:
                for h in range(2):
                    hs = slice(h * half, (h + 1) * half)
                    nc.vector.tensor_scalar(bt[:, hs], b_ap[:, hs],
                                            col(8 + p), None, op0=ALU.add)
            if p < 3:
                nc.vector.tensor_tensor(pt[p][:], at[:], bt[:], op=ALU.mult)
            else:
                for h in range(2):
                    hs = slice(h * half, (h + 1) * half)
                    nc.vector.tensor_tensor(pt[p][:, hs], at[:, hs],
                                            bt[:, hs], op=ALU.mult)

        xq0 = tmp.tile([128, FD], F16, tag="x", name="xq0")
        xq1 = tmp.tile([128, FD], F16, tag="x2", name="xq1")
        yq0 = tmp.tile([128, FD], F16, tag="y", name="yq0")
        yq1 = tmp.tile([128, FD], F16, tag="y2", name="yq1")
        xq = [xq0, xq1]
        yq = [yq0, yq1]
        for q in range(2):
            nc.vector.tensor_scalar(xq[q][:], pt[2 * q][:], col(12 + q),
                                    col(14 + q), op0=ALU.mult, op1=ALU.add)
        nc.scalar.activation(yq[0][:], pt[1][:], AF.Identity, bias=col(16),
                             scale=1.0)
        nc.vector.tensor_tensor(mt[0][:], xq[0][:], yq[0][:], op=ALU.mult)

        xr = tmp.tile([128, FD], F16, tag="x")
        yr = tmp.tile([128, FD], F16, tag="y")
        rt = tmp.tile([128, FD], F16, tag="a")
        nc.vector.tensor_scalar(xr[:], mt[0][:], col(18), col(19),
                                op0=ALU.mult, op1=ALU.add)
        # leaf7-dependent chain runs in free-dim halves to shorten the tail
        for h in range(2):
            hs = slice(h * half, (h + 1) * half)
            nc.vector.tensor_scalar(yq[1][:, hs], pt[3][:, hs], col(17),
                                    None, op0=ALU.add)
            nc.vector.tensor_tensor(mt[1][:, hs], xq[1][:, hs], yq[1][:, hs],
                                    op=ALU.mult)
            nc.vector.tensor_scalar(yr[:, hs], mt[1][:, hs], col(20), None,
                                    op0=ALU.add)
            nc.vector.tensor_tensor(rt[:, hs], xr[:, hs], yr[:, hs],
                                    op=ALU.mult)
            nc.vector.tensor_scalar(ot[:, hs], rt[:, hs], col(21), None,
                                    op0=ALU.add)
            swq(nc.gpsimd.dma_start(
                out=out_ext[:].rearrange("o b f -> o (b f)")[:, hs],
                in_=ot[:, hs],
            ))

    nc.compile()
    return nc


def _softmax64(w):
    e = np.exp(w - w.max(axis=-1, keepdims=True))
    return e / e.sum(axis=-1, keepdims=True)


def make_host_inputs(x, weights, leaf_indices):
    """Shared input prep: per-core in_maps (kernel shards batch over cores)."""
    x = np.asarray(x, dtype=np.float32)  # cast to fp16 per-core below
    weights = np.asarray(weights, dtype=np.float64)
    leaf_indices = np.asarray(leaf_indices).astype(np.int64)  # [C_OUT, 8]

    # ---- gather row offsets: u row = c*9 + tap, one row per (o, leaf)
    c = leaf_indices // 9
    tap = leaf_indices % 9
    gidx = np.ascontiguousarray(c * 9 + tap, dtype=np.int32)  # [128, 8]

    # ---- per-node factored coefficients in f64
    km = _softmax64(weights) @ GATE_C  # [128, 7, 4] -> k0, ka, kb, kab
    def coef(r):
        k0, ka, kb, kab = (km[:, r, i] for i in range(4))
        return kb / kab, ka / kab, k0 - ka * kb / kab, kab  # alpha, beta, delta

    a0, b0, d0, kab0 = zip(*[coef(r) for r in L0_ROWS])
    a1, b1, d1, kab1 = zip(*[coef(r) for r in L1_ROWS])
    aR, bR, dR, kabR = coef(L2_ROW)

    # column layout:
    #  0..3  kab0_p | 4..7  kb0_p (=kab0*alpha0) | 8..11 beta0_p
    # 12..13 kab1_q | 14..15 kab1_q*(d0_{2q}+a1_q) | 16..17 d0_{2q+1}+b1_q
    # 18 kabR | 19 kabR*(d1_0+aR) | 20 d1_1+bR | 21 dR
    sc = np.zeros((128, N_SC), np.float64)
    for p in range(4):
        sc[:, p] = kab0[p]
        sc[:, 4 + p] = kab0[p] * a0[p]
        sc[:, 8 + p] = b0[p]
    for q in range(2):
        sc[:, 12 + q] = kab1[q]
        sc[:, 14 + q] = kab1[q] * (d0[2 * q] + a1[q])
        sc[:, 16 + q] = d0[2 * q + 1] + b1[q]
    sc[:, 18] = kabR
    sc[:, 19] = kabR * (d1[0] + aR)
    sc[:, 20] = d1[1] + bR
    sc[:, 21] = dR
    sc = np.ascontiguousarray(sc, dtype=np.float32)

    in_maps = []
    for core in range(N_CORES):
        # channel-partition layout: [C_IN, (b, h, w)]
        xs = np.ascontiguousarray(
            x[core * NB:(core + 1) * NB].transpose(1, 0, 2, 3).reshape(C_IN, -1),
            dtype=np.float16,
        )
        in_maps.append({"x": xs, "sc": sc, "gidx": gidx})
    return in_maps


_NC_CACHE = {}


def kernel(x, weights, leaf_indices):
    key = "prog"
    if key not in _NC_CACHE:
        _NC_CACHE[key] = build_program()
    nc = _NC_CACHE[key]
    in_maps = make_host_inputs(x, weights, leaf_indices)
    res = run_bass_kernel_spmd(nc, in_maps, list(range(N_CORES)))
    out = np.concatenate(
        [r["out"].astype(np.float32).reshape(C_OUT, NB, H, W).transpose(1, 0, 2, 3)
         for r in res.results], axis=0
    )
    return out
